# revision 17
# baseline (speedup 1.0000x reference)
"""Trainium2 Bass kernel for nn_Block_self_attention_inter_intra_3D.

Math: the reference loops 36 overlapping windows (i,j in 0..2, z in 0..3) of a
(2,64,48,48,16) volume, runs channel-projected position attention inside each
window (reading the ORIGINAL x), and writes results back last-write-wins.
Because windows are boxes and later windows overwrite earlier ones, each window
"owns" exactly its local [0:16,0:16,0:4] sub-box (1024 positions) of the
output.  So per window we need attention only for those 1024 query positions
against all N window key positions (N in {3456,2304,1536,1024}).

Sharding: 72 (window,batch) tasks -> 8 cores x 9 uniform slots
(3x N=3456, 4x N=2304, 2x N=1536).  The two N=1024 tasks land on core 7 padded
to 1536 with an additive -1e9 key mask folded in as a 65th contraction channel.
Every core runs the identical program (SPMD) on its own slot data.

Per-task device pipeline (keys m on PSUM partitions so softmax needs no
transposes).  Both q- and k-projections are fused into ONE host-precomputed
projection of the queries, and the v-projection is eliminated by factoring
att@v through the raw inputs (G-trick), so per m-tile of 128 keys the device
does only: 2 energy matmuls -> 1 exp -> 2 accumulate matmuls:
  qk_aug = A^T xq + c      A = Wk^T Wq, c = Wk^T bq  (+ row 64 = bk^T q,
                           row 65 = 1 to activate the mask channel)
  energyT[m,n] = xk66[:,m]^T qk_aug[:,n]   xk66 = [x window; ones; mask] comes
                           straight from DRAM -- k is never materialized
  expE = exp(energyT)      (no max-subtraction: |energy| <~ 50, safe in fp32)
  G_aug = xkT_aug @ expE   accumulated over m-tiles in PSUM; row 64 = sum(exp)
  out_unnorm = Wv_aug^T @ G_aug      (bias bv enters via G's sum(exp) row)
  out = out_unnorm / sumexp + xq     (1/sumexp broadcast across partitions via
                                      a K=1 ones matmul, then DVE mul/add)
Matmuls run as float32r (TF32-like, full PE rate at free dim >=256); the
ScalarE exp stream (~1.04us per 128x1024 m-tile) is the critical resource and
the schedule keeps it >95% busy in steady state.
"""

import os
import sys

sys.path.insert(0, "/opt/trn_rl_repo")

from contextlib import ExitStack

import numpy as np

import concourse.bacc as bacc
import concourse.mybir as mybir
import concourse.tile as tile
from concourse.bass_utils import run_bass_kernel_spmd

F32 = mybir.dt.float32
F32R = mybir.dt.float32r

N_CORES = 8
NQ = 1024
SLOT_NK = [3456, 3456, 3456, 2304, 2304, 2304, 2304, 1536, 1536]
MASK_NEG = -1.0e9

B, C, H, W, T = 2, 64, 48, 48, 16


def _win(i):
    s = 16 * i
    return s, min(s + 24, 48) - s


def _win_z(z):
    s = 4 * z
    return s, min(s + 6, 16) - s


def _task_lists():
    t3456 = [(b, i, j, z) for b in (0, 1) for i in (0, 1) for j in (0, 1)
             for z in (0, 1, 2)]
    t2304 = ([(b, i, j, 3) for b in (0, 1) for i in (0, 1) for j in (0, 1)] +
             [(b, i, 2, z) for b in (0, 1) for i in (0, 1) for z in (0, 1, 2)] +
             [(b, 2, j, z) for b in (0, 1) for j in (0, 1) for z in (0, 1, 2)])
    t1536 = ([(b, i, 2, 3) for b in (0, 1) for i in (0, 1)] +
             [(b, 2, j, 3) for b in (0, 1) for j in (0, 1)] +
             [(b, 2, 2, z) for b in (0, 1) for z in (0, 1, 2)])
    t1024 = [(b, 2, 2, 3) for b in (0, 1)]
    assert len(t3456) == 24 and len(t2304) == 32
    assert len(t1536) == 14 and len(t1024) == 2
    tail = t1536 + t1024
    per_core = []
    for c in range(N_CORES):
        per_core.append(t3456[3 * c:3 * c + 3] + t2304[4 * c:4 * c + 4] +
                        tail[2 * c:2 * c + 2])
    return per_core


TASKS = _task_lists()


def _emit(nc, tc, ctx, aps, reps):
    wp = ctx.enter_context(tc.tile_pool(name="wp", bufs=1))
    sb = ctx.enter_context(tc.tile_pool(name="sb", bufs=2))
    expp = ctx.enter_context(tc.tile_pool(name="expp", bufs=3))
    pse = ctx.enter_context(tc.tile_pool(name="pse", bufs=2, space="PSUM"))
    psm = ctx.enter_context(tc.tile_pool(name="psm", bufs=2, space="PSUM"))
    pso = ctx.enter_context(tc.tile_pool(name="pso", bufs=1, space="PSUM"))

    Exp = mybir.ActivationFunctionType.Exp

    # all weights arrive in one packed DMA (DMA dispatch slots are ~0.65us
    # each, so count matters on the startup critical path):
    # [0:64,0:66] A_lhsT (fused Wq^T Wk | Wq^T bk | 0) |
    # [64,0:64] ones row (for the 1/sumexp broadcast, partition 64) |
    # [0:65,66:130] Wv_aug | [0:66,130] b66 = [Wk^T bq ; bk.bq ; 1]
    wpk = wp.tile([128, 131], F32R, tag="wpk")
    nc.sync.dma_start(wpk[:], aps["wpk"][:])
    alb = wpk[0:64, 0:66]
    one_row = wpk[64:65, 0:64]
    wva = wpk[0:65, 66:130]
    b66 = wpk[0:66, 130:131].bitcast(F32)

    def proj_gen(s, chunked=False):
        """Projection phase for slot s, as a generator so its PE work can
        be emitted interleaved with the previous slot's attention m-loop
        (filling PE slack while the ScalarE exp stream stays hot).
        chunked=True DMAs xk in 512-col pieces so the very first k-proj
        (and hence the exp stream) starts as soon as possible."""
        nk = SLOT_NK[s]
        mt = nk // 128
        st = {}
        # xk arrives with 66 rows: 0:64 = channels, 64 = ones (adds bk.q
        # per valid key), 65 = additive mask -- and is used DIRECTLY as
        # the energy stationary operand: no k-projection on device, since
        # energyT = xk66^T @ (A xq + c) with A = Wk^T Wq fused on host.
        xk = sb.tile([66, nk], F32R, tag="xk")
        if not chunked:
            nc.sync.dma_start(xk[:], aps[f"xk{s}"][:])
        xq = sb.tile([64, NQ], F32R, tag="xq")
        nc.sync.dma_start(xq[:], aps[f"xq{s}"][:])
        # xkT (key-major, with ones col 64) comes precomputed from the host;
        # it is both the att@v stationary operand and the source of the
        # sum(exp) row (G-trick: att@v = Wv_aug @ (xkT_aug @ exp)).
        xkt = sb.tile([128, mt * 65], F32R, tag="xkt")
        if not chunked:
            nc.sync.dma_start(xkt[:], aps[f"xkt{s}"][:])
        xkt3 = xkt[:].rearrange("p (t c) -> p t c", c=65)
        qag = sb.tile([66, NQ], F32R, tag="qag")
        st.update(s=s, nk=nk, mt=mt, xk=xk, xq=xq, xkt3=xkt3, qag=qag)
        yield st

        if chunked:
            off = 0
            xkt_off = 0
            while off < nk:
                w = min(512, nk - off)
                nc.sync.dma_start(xk[:, off:off + w],
                                  aps[f"xk{s}"][:, off:off + w])
                pw = min(4 * 65, mt * 65 - xkt_off)
                if pw > 0:
                    nc.sync.dma_start(
                        xkt[:, xkt_off:xkt_off + pw],
                        aps[f"xkt{s}"][:, xkt_off:xkt_off + pw])
                    xkt_off += pw
                off += w

        # fused qk projection: qk_aug = A_lhsT^T xq (+ b66 incl the ones
        # row that turns the mask channel on)
        for o in (0, 512):
            qps = psm.tile([128, 512], F32, tag="m")
            nc.tensor.matmul(qps[0:66, :], alb, xq[:, o:o + 512],
                             start=True, stop=True)
            nc.vector.tensor_scalar_add(qag[:, o:o + 512], qps[0:66, :],
                                        b66)
            yield st

    def drain(gen):
        if gen is not None:
            for _ in gen:
                pass

    def mloop(st, nxt_gen):
        """Attention m-loop for a prepared slot; advances nxt_gen (the next
        slot's projection generator) between m-tiles."""
        s, nk, mt = st["s"], st["nk"], st["mt"]
        xk, qag, xkt3, xq = st["xk"], st["qag"], st["xkt3"], st["xq"]
        ops = pso.tile([65, NQ], F32, tag="o")

        def _energy(t):
            eps = pse.tile([128, NQ], F32, tag="e")
            lhs = xk[:, t * 128:(t + 1) * 128]
            nc.tensor.matmul(eps[:, 0:512], lhs, qag[:, 0:512],
                             start=True, stop=True)
            nc.tensor.matmul(eps[:, 512:1024], lhs, qag[:, 512:1024],
                             start=True, stop=True)
            return eps

        eps = _energy(0)
        for t in range(mt):
            ex = expp.tile([128, NQ], F32R, tag="ex")
            nc.scalar.activation(ex[:], eps[:], Exp)
            if t + 1 < mt:
                eps = _energy(t + 1)
            vt = xkt3[:, t, :]
            nc.tensor.matmul(ops[:, 0:512], vt, ex[:, 0:512],
                             start=(t == 0), stop=(t == mt - 1),
                             skip_group_check=True)
            nc.tensor.matmul(ops[:, 512:1024], vt, ex[:, 512:1024],
                             start=(t == 0), stop=(t == mt - 1),
                             skip_group_check=True)
            if nxt_gen is not None:
                next(nxt_gen, None)

        drain(nxt_gen)
        # ops now holds G_aug: rows 0:64 = xk @ exp (per in-channel),
        # row 64 = sum(exp).  out_unnorm = Wv_aug^T @ G_aug (bias enters
        # via G row 64 since Wv_aug row 64 = bv).
        ocn = sb.tile([65, NQ], F32R, tag="ocn")
        nc.vector.tensor_copy(ocn[:], ops[:])
        rec = sb.tile([64, NQ], F32, tag="rec")
        tmp = sb.tile([64, NQ], F32, tag="tmp")
        for o in (0, 512):
            # broadcast 1/sumexp across 64 partitions via K=1 matmul
            # (`one` lives on partition 64, matching ocn's sumexp row)
            bps = psm.tile([128, 512], F32, tag="m")
            nc.tensor.matmul(bps[0:64, :], one_row,
                             ocn[64:65, o:o + 512], start=True, stop=True)
            nc.vector.reciprocal(rec[:, o:o + 512], bps[0:64, :])
            fps = psm.tile([128, 512], F32, tag="m")
            nc.tensor.matmul(fps[0:64, :], wva, ocn[:, o:o + 512],
                             start=True, stop=True)
            nc.vector.tensor_mul(tmp[:, o:o + 512], fps[0:64, :],
                                 rec[:, o:o + 512])
        fin = sb.tile([64, NQ], F32, tag="fin")
        nc.vector.tensor_add(fin[:], tmp[:], xq[:])
        nc.sync.dma_start(aps["o"][s], fin[:])

    # smallest slot first so the exp stream warms up quickly
    order = [7, 0, 1, 2, 3, 4, 5, 6, 8] * reps
    gen = proj_gen(order[0], chunked=True)
    st = next(gen)
    for idx in range(len(order)):
        if idx == 0:
            # first slot: emit all its projections (and their chunked
            # DMAs) before the next slot's big DMAs enter the queues
            drain(gen)
        if idx + 1 < len(order):
            ngen = proj_gen(order[idx + 1])
            nst = next(ngen)
        else:
            ngen, nst = None, None
        # finish this slot's own projections before its m-loop
        drain(gen)
        mloop(st, ngen)
        gen, st = ngen, nst


_CACHE = {}


def _build(reps):
    if reps in _CACHE:
        return _CACHE[reps]
    nc = bacc.Bacc("TRN2", target_bir_lowering=False, debug=False,
                   enable_asserts=True)
    aps = {}
    for s, nk in enumerate(SLOT_NK):
        aps[f"xk{s}"] = nc.dram_tensor(f"xk{s}", [66, nk], F32R,
                                       kind="ExternalInput").ap()
        aps[f"xq{s}"] = nc.dram_tensor(f"xq{s}", [64, NQ], F32R,
                                       kind="ExternalInput").ap()
        aps[f"xkt{s}"] = nc.dram_tensor(f"xkt{s}", [128, (nk // 128) * 65],
                                        F32R, kind="ExternalInput").ap()
    aps["wpk"] = nc.dram_tensor("wpk", [128, 131], F32R,
                                kind="ExternalInput").ap()
    aps["o"] = nc.dram_tensor("o", [9, 64, NQ], F32, kind="ExternalOutput").ap()

    with tile.TileContext(nc) as tc:
        with ExitStack() as ctx:
            _emit(nc, tc, ctx, aps, reps)
    nc.compile()
    _CACHE[reps] = nc
    return nc


def _host_inputs(x, Wq, bq, Wk, bk, Wv, bv):
    x = np.asarray(x, np.float32)
    Wq = np.asarray(Wq, np.float32)
    Wk = np.asarray(Wk, np.float32)
    Wv = np.asarray(Wv, np.float32)
    bq = np.asarray(bq, np.float32)
    bk = np.asarray(bk, np.float32)
    bv = np.asarray(bv, np.float32)

    wpk = np.zeros((128, 131), np.float32)
    wpk[0:64, 0:64] = Wq.T @ Wk     # A_lhsT = (Wk^T Wq)^T
    wpk[0:64, 64] = Wq.T @ bk
    wpk[64, 0:64] = 1.0             # ones row for the 1/sumexp broadcast
    wpk[0:65, 66:130] = np.concatenate([Wv.T, bv[None, :]], axis=0)
    wpk[0:64, 130] = Wk.T @ bq
    wpk[64, 130] = bk @ bq
    wpk[65, 130] = 1.0              # turns the mask channel on
    shared = {"wpk": wpk}
    in_maps = []
    for c in range(N_CORES):
        m = dict(shared)
        for s, (b, i, j, z) in enumerate(TASKS[c]):
            nk_slot = SLOT_NK[s]
            sx, dx = _win(i)
            sy, dy = _win(j)
            sz, dz = _win_z(z)
            win = x[b, :, sx:sx + dx, sy:sy + dy, sz:sz + dz]
            nk = dx * dy * dz
            xkb = np.zeros((66, nk_slot), np.float32)
            xkb[0:64, :nk] = win.reshape(64, nk)
            xkb[64, :nk] = 1.0
            xkb[65, nk:] = MASK_NEG
            m[f"xk{s}"] = xkb
            mt = nk_slot // 128
            m[f"xkt{s}"] = np.ascontiguousarray(
                xkb[0:65].reshape(65, mt, 128).transpose(2, 1, 0)
                .reshape(128, mt * 65))
            m[f"xq{s}"] = np.ascontiguousarray(
                win[:, 0:16, 0:16, 0:4].reshape(64, NQ))
        in_maps.append(m)
    return in_maps


def _scatter(results):
    out = np.empty((B, C, H, W, T), np.float32)
    for c in range(N_CORES):
        o = results[c]["o"]
        for s, (b, i, j, z) in enumerate(TASKS[c]):
            sx, _ = _win(i)
            sy, _ = _win(j)
            sz, _ = _win_z(z)
            blk = o[s].reshape(64, 16, 16, 4)
            out[b, :, sx:sx + 16, sy:sy + 16, sz:sz + 4] = blk
    return out


def _ensure_axon():
    # The axon PJRT plugin is registered by sitecustomize at interpreter
    # start; if a caller pinned JAX_PLATFORMS=cpu before jax init, try to
    # re-enable the axon backend (run_bass_via_pjrt needs 8 trn2 devices).
    import jax

    try:
        if any(d.platform == "axon" for d in jax.devices()):
            return
    except Exception:
        pass
    try:
        jax.config.update("jax_platforms", "axon,cpu")
        jax.extend.backend.clear_backends()
    except Exception:
        pass


def run(x, Wq, bq, Wk, bk, Wv, bv, reps=1):
    _ensure_axon()
    nc = _build(reps)
    in_maps = _host_inputs(x, Wq, bq, Wk, bk, Wv, bv)
    res = run_bass_kernel_spmd(nc, in_maps, core_ids=list(range(N_CORES)))
    return _scatter(res.results), res


def kernel(x, Wq, bq, Wk, bk, Wv, bv):
    out, _ = run(x, Wq, bq, Wk, bk, Wv, bv,
                 reps=int(os.environ.get("KREP", "1")))
    return out
